# revision 1
# baseline (speedup 1.0000x reference)
"""8x8 block DCT (DCT-II) on [64,1,1024,1024] fp32 -> [64,64,128,128].

Data parallel over batch: 8 images per NeuronCore on 8 cores.

Per 128x128 image tile T, the 2D DCT of all 256 8x8 blocks is two dense
PE matmuls against one constant block-diagonal permuted DCT matrix DT1
(DT1[8*b + x, 16*u + b] = M[u, x]):
    U = T^T @ DT1        [c, 16u+bi]     (stage 1, fp32)
    Z = U^T @ DT1        [16u+bi, 16v+bj] (stage 2, fp16 hi/lo x3, ~1e-6 rel)
Stage 2 splits U into fp16 hi+lo during the mandatory PSUM drain and uses
fp16 hi/lo DCT constants, accumulating three fp16 matmuls in PSUM: full
fp32-grade accuracy at 1 cycle/row instead of 4.

Z is scatter-drained into a per-image SBUF buffer laid out [p=16u+bi,
f = v*1024 + ti*128 + J] so each (img, u) stores with ONE 512KB DMA whose
3-dim AP covers 8 output channels. Output descriptors are 512B (forced:
block-row index bi lives on partitions); throughput recovers by spreading
descriptor generation across the three DGE paths (SP-HWDGE, ACT-HWDGE,
GPSIMD-SWDGE).
"""

import numpy as np

_N_CORES = 8
_H = 1024
_W = 1024

_NC_CACHE = {}

# tuning knobs
OUT_ENGINES = "sscg"  # cycle pattern: s=sync, c=scalar, g=gpsimd
IN_ENGINE = "g"
GROUP = 4  # tiles per PSUM bank group (must divide 8)
SCATTER_SPLIT = True
ZIMG_BUFS = 3
XS_BUFS = 3
HOST_SPLIT = False


def _dct_mat_np():
    n = 8
    u = np.arange(n)[:, None].astype(np.float64)
    x = np.arange(n)[None, :].astype(np.float64)
    m = np.cos((2 * x + 1) * u * np.pi / (2 * n))
    scale = np.where(u == 0, np.sqrt(1.0 / n), np.sqrt(2.0 / n))
    return (m * scale).astype(np.float32)


def _build_dt1(dct: np.ndarray) -> np.ndarray:
    """DT1[8*b + x, 16*u + b] = dct[u, x], zero elsewhere."""
    dt1 = np.zeros((128, 128), dtype=np.float32)
    for b in range(16):
        dt1[8 * b : 8 * b + 8, b::16] = dct.T
    return dt1


def build_nc(
    n_img: int,
    out_engines=OUT_ENGINES,
    in_engine=IN_ENGINE,
    group=GROUP,
    scatter_split=SCATTER_SPLIT,
    zimg_bufs=ZIMG_BUFS,
    xs_bufs=XS_BUFS,
    strip_input=False,
    host_split=HOST_SPLIT,
):
    import concourse.bacc as bacc
    import concourse.mybir as mybir
    import concourse.tile as tile

    f32 = mybir.dt.float32
    f16 = mybir.dt.float16
    nc = bacc.Bacc("TRN2", target_bir_lowering=False, debug=False)

    if host_split:
        x = nc.dram_tensor("x", [n_img, 1, _H, 2 * _W], f16, kind="ExternalInput")
    else:
        x = nc.dram_tensor("x", [n_img, 1, _H, _W], f32, kind="ExternalInput")
    dt1 = nc.dram_tensor("dt1", [128, 128], f32, kind="ExternalInput")
    dt1h = nc.dram_tensor("dt1h", [128, 128], f16, kind="ExternalInput")
    dt1l = nc.dram_tensor("dt1l", [128, 128], f16, kind="ExternalInput")
    out = nc.dram_tensor("out", [n_img, 64, 128, 128], f32, kind="ExternalOutput")

    def eng(ch):
        return {"s": nc.sync, "c": nc.scalar, "g": nc.gpsimd}[ch]

    n_out_dma = 0

    with tile.TileContext(nc) as tc:
        with (
            tc.tile_pool(name="const", bufs=1) as constp,
            tc.tile_pool(
                name="xs", bufs=(xs_bufs * 8 if strip_input else xs_bufs)
            ) as xsp,
            tc.tile_pool(name="zimg", bufs=zimg_bufs) as zp,
            tc.tile_pool(name="uhi", bufs=3) as uhip,
            tc.tile_pool(name="ulo", bufs=3) as ulop,
            tc.tile_pool(name="psu", bufs=(3 if group <= 4 else 2), space="PSUM") as psu,
            tc.tile_pool(name="psz", bufs=(3 if group <= 4 else 2), space="PSUM") as psz,
        ):
            dt1_t = constp.tile([128, 128], f32)
            nc.sync.dma_start(dt1_t[:], dt1[:])
            dt1h_t = constp.tile([128, 128], f16)
            nc.sync.dma_start(dt1h_t[:], dt1h[:])
            dt1l_t = constp.tile([128, 128], f16)
            nc.sync.dma_start(dt1l_t[:], dt1l[:])

            for img in range(n_img):
                if host_split:
                    # xs[p, s*2048 + c] = x[img, 0, 128*s+p, c]; row = hi|lo
                    xs = xsp.tile([128, 8 * 2 * _W], f16)
                    src = x[img, 0, :, :].rearrange("(s p) c -> p s c", p=128)
                    eng(in_engine).dma_start(
                        xs[:].rearrange("p (s c) -> p s c", s=8), src
                    )
                elif strip_input:
                    xstrips = []
                    for ti in range(8):
                        xst = xsp.tile([128, _W], f32, tag="xstrip")
                        eng(in_engine).dma_start(
                            xst[:], x[img, 0, 128 * ti : 128 * (ti + 1), :]
                        )
                        xstrips.append(xst)
                else:
                    # Load full image: xs[p, s*1024 + c] = x[img, 0, 128*s+p, c]
                    xs = xsp.tile([128, 8 * _W], f32)
                    src = x[img, 0, :, :].rearrange("(s p) c -> p s c", p=128)
                    eng(in_engine).dma_start(
                        xs[:].rearrange("p (s c) -> p s c", s=8), src
                    )

                # Zimg[p=16u+bi, v*1024 + ti*128 + tj*16 + bj]
                zimg = zp.tile([128, 8 * _W], f32)

                for ti in range(8):
                    for tj0 in range(0, 8, group):
                        gw = group * 128
                        u_ps = psu.tile([128, gw], f32)
                        for q in range(group):
                            tj = tj0 + q
                            uq = u_ps[:, q * 128 : (q + 1) * 128]
                            if host_split:
                                hi = xs[
                                    :,
                                    ti * 2048 + tj * 128 : ti * 2048 + (tj + 1) * 128,
                                ]
                                lo = xs[
                                    :,
                                    ti * 2048 + 1024 + tj * 128 : ti * 2048
                                    + 1024
                                    + (tj + 1) * 128,
                                ]
                                nc.tensor.matmul(
                                    uq, hi, dt1h_t[:], start=True, stop=False
                                )
                                nc.tensor.matmul(
                                    uq, hi, dt1l_t[:], start=False, stop=False
                                )
                                nc.tensor.matmul(
                                    uq, lo, dt1h_t[:], start=False, stop=True
                                )
                                continue
                            if strip_input:
                                lhs = xstrips[ti][:, tj * 128 : (tj + 1) * 128]
                            else:
                                lhs = xs[
                                    :,
                                    ti * 1024 + tj * 128 : ti * 1024 + (tj + 1) * 128,
                                ]
                            nc.tensor.matmul(
                                uq,
                                lhs,
                                dt1_t[:],
                                start=True,
                                stop=True,
                            )
                        u_hi = uhip.tile([128, gw], f16)
                        nc.scalar.copy(u_hi[:], u_ps[:])
                        u_lo = ulop.tile([128, gw], f16)
                        nc.vector.tensor_sub(u_lo[:], u_ps[:], u_hi[:])

                        z_ps = psz.tile([128, gw], f32)
                        for q in range(group):
                            zq = z_ps[:, q * 128 : (q + 1) * 128]
                            hi_q = u_hi[:, q * 128 : (q + 1) * 128]
                            lo_q = u_lo[:, q * 128 : (q + 1) * 128]
                            nc.tensor.matmul(
                                zq, hi_q, dt1h_t[:], start=True, stop=False
                            )
                            nc.tensor.matmul(
                                zq, hi_q, dt1l_t[:], start=False, stop=False
                            )
                            nc.tensor.matmul(
                                zq, lo_q, dt1h_t[:], start=False, stop=True
                            )

                        # scatter: z_ps[p, q*128 + 16v + bj]
                        #   -> zimg[p, v*1024 + ti*128 + (tj0+q)*16 + bj]
                        src4 = z_ps[:].rearrange("p (q v b) -> p q v b", q=group, v=8)
                        dstv = zimg[:].rearrange(
                            "p (v t j) -> p v t j", v=8, t=8
                        )[:, :, ti, tj0 * 16 : tj0 * 16 + group * 16]
                        dst4 = dstv.rearrange("p v (q b) -> p q v b", q=group)
                        if scatter_split and (ti * (8 // group) + tj0 // group) % 2:
                            nc.scalar.copy(dst4, src4)
                        else:
                            nc.vector.tensor_copy(dst4, src4)

                # Store: one fat DMA per u covering channels 8u..8u+8
                for u in range(8):
                    src = zimg[16 * u : 16 * u + 16, :]
                    dst = out[img, 8 * u : 8 * u + 8, :, :].rearrange(
                        "v (t b) j -> b (v t) j", b=16
                    )
                    e = out_engines[n_out_dma % len(out_engines)]
                    n_out_dma += 1
                    eng(e).dma_start(dst, src)

    nc.compile()
    return nc


def _get_nc(n_img: int):
    if n_img not in _NC_CACHE:
        _NC_CACHE[n_img] = build_nc(n_img)
    return _NC_CACHE[n_img]


def _split_f16(m: np.ndarray):
    hi = m.astype(np.float16)
    lo = (m - hi.astype(np.float32)).astype(np.float16)
    return hi, lo


def make_inputs(x_core: np.ndarray, dct: np.ndarray, host_split=False) -> dict:
    dt1 = _build_dt1(dct)
    dt1h, dt1l = _split_f16(dt1)
    if host_split:
        xh = x_core.astype(np.float16)
        xl = (x_core - xh.astype(np.float32)).astype(np.float16)
        x_core = np.concatenate((xh, xl), axis=-1)
    return {"x": x_core, "dt1": dt1, "dt1h": dt1h, "dt1l": dt1l}


def run_spmd(
    x: np.ndarray, dct: np.ndarray, trace: bool = False, nc=None, host_split=HOST_SPLIT
):
    """Run the SPMD kernel on 8 cores. Returns (out, BassKernelResults)."""
    from concourse.bass_utils import run_bass_kernel_spmd

    x = np.ascontiguousarray(np.asarray(x, dtype=np.float32))
    dct = np.asarray(dct, dtype=np.float32)
    b = x.shape[0]
    per = b // _N_CORES

    if nc is None:
        nc = _get_nc(per)
    in_maps = [
        make_inputs(x[i * per : (i + 1) * per], dct, host_split=host_split)
        for i in range(_N_CORES)
    ]
    res = run_bass_kernel_spmd(
        nc, in_maps, core_ids=list(range(_N_CORES)), trace=trace
    )
    out = np.concatenate(
        [res.results[i]["out"] for i in range(_N_CORES)], axis=0
    )
    return out, res


def kernel(x, dct=None):
    if dct is None:
        dct = _dct_mat_np()
    out, _ = run_spmd(x, dct, trace=False)
    return out



# revision 2
# speedup vs baseline: 1.8169x; 1.8169x over previous
"""8x8 block DCT (DCT-II) on [64,1,1024,1024] fp32 -> [64,64,128,128].

Data parallel over batch: 8 images per NeuronCore on 8 cores.

Accuracy budget is rel_err < 2e-2 (vs max|out|), so the whole pipeline
runs in fp16 (measured rel err ~4e-4):
  - host casts x to fp16 (halves input HBM traffic),
  - both DCT stages are single fp16 matmuls (1 cycle/row on PE vs 4 for
    fp32) against the block-diagonal permuted DCT constant DT1
    (DT1[8*b + x, 16*u + b] = M[u, x]),
  - stage-2 results are drained PSUM->SBUF as fp16 and DMA'd to DRAM in
    the compute-native layout (contiguous 2KB runs per partition), and
    the host applies the fixed (img,u,v,ti,bi,tjg,q,bj) permutation +
    fp32 upcast.

Per 128x128 image tile T:
    U = T^T @ DT1        [c, 16u+bi]       (stage 1)
    Z = U^T @ DT1        [16u+bi, 16v+bj]  (stage 2)
Device output zraw[img, ti, p=16u+bi, tjg*512 + q*128 + 16v + bj] with
tj = 4*tjg + q; true output out[img, 8u+v, 16ti+bi, 16tj+bj].
"""

import numpy as np

_N_CORES = 8
_H = 1024
_W = 1024

_NC_CACHE = {}

# tuning knobs
OUT_ENGINES = "ssc"  # cycle per out-DMA: s=sync, c=scalar, g=gpsimd, v=vector
IN_ENGINE = "g"  # cycle per image in-DMA
U_DRAIN = "c"  # engine(s) for stage-1 PSUM drain
Z_DRAIN = "v"  # engine(s) for stage-2 PSUM drain
XS_BUFS = 3
Z_BUFS = 3


def _dct_mat_np():
    n = 8
    u = np.arange(n)[:, None].astype(np.float64)
    x = np.arange(n)[None, :].astype(np.float64)
    m = np.cos((2 * x + 1) * u * np.pi / (2 * n))
    scale = np.where(u == 0, np.sqrt(1.0 / n), np.sqrt(2.0 / n))
    return (m * scale).astype(np.float32)


def _build_dt1(dct: np.ndarray) -> np.ndarray:
    """DT1[8*b + x, 16*u + b] = dct[u, x], zero elsewhere."""
    dt1 = np.zeros((128, 128), dtype=np.float32)
    for b in range(16):
        dt1[8 * b : 8 * b + 8, b::16] = dct.T
    return dt1


def build_nc(
    n_img: int,
    out_engines=OUT_ENGINES,
    in_engine=IN_ENGINE,
    u_drain=U_DRAIN,
    z_drain=Z_DRAIN,
    xs_bufs=XS_BUFS,
    z_bufs=Z_BUFS,
):
    import concourse.bacc as bacc
    import concourse.mybir as mybir
    import concourse.tile as tile

    f32 = mybir.dt.float32
    f16 = mybir.dt.float16
    nc = bacc.Bacc("TRN2", target_bir_lowering=False, debug=False)

    x = nc.dram_tensor("x", [n_img, 1, _H, _W], f16, kind="ExternalInput")
    dt1h = nc.dram_tensor("dt1h", [128, 128], f16, kind="ExternalInput")
    zraw = nc.dram_tensor(
        "zraw", [n_img, 8, 128, 1024], f16, kind="ExternalOutput"
    )

    def eng(ch):
        return {
            "s": nc.sync,
            "c": nc.scalar,
            "g": nc.gpsimd,
            "v": nc.vector,
        }[ch]

    def copy_on(ch, dst, src):
        if ch == "v":
            nc.vector.tensor_copy(dst, src)
        else:
            eng(ch).copy(dst, src)

    n_in = 0
    n_out = 0
    n_ud = 0
    n_zd = 0

    with tile.TileContext(nc) as tc:
        with (
            tc.tile_pool(name="const", bufs=1) as constp,
            tc.tile_pool(name="xs", bufs=xs_bufs) as xsp,
            tc.tile_pool(name="zt", bufs=z_bufs) as ztp,
            tc.tile_pool(name="u16", bufs=3) as u16p,
            tc.tile_pool(name="psu", bufs=3, space="PSUM") as psu,
            tc.tile_pool(name="psz", bufs=3, space="PSUM") as psz,
        ):
            dt1h_t = constp.tile([128, 128], f16)
            nc.sync.dma_start(dt1h_t[:], dt1h[:])

            for img in range(n_img):
                # xs[p, s*1024 + c] = x[img, 0, 128*s+p, c]
                xs = xsp.tile([128, 8 * _W], f16)
                src = x[img, 0, :, :].rearrange("(s p) c -> p s c", p=128)
                e = in_engine[n_in % len(in_engine)]
                n_in += 1
                eng(e).dma_start(xs[:].rearrange("p (s c) -> p s c", s=8), src)

                for ti in range(8):
                    zt = ztp.tile([128, 1024], f16)
                    for tjg in range(2):
                        u_ps = psu.tile([128, 512], f32)
                        for q in range(4):
                            tj = 4 * tjg + q
                            nc.tensor.matmul(
                                u_ps[:, q * 128 : (q + 1) * 128],
                                xs[:, ti * 1024 + tj * 128 : ti * 1024 + (tj + 1) * 128],
                                dt1h_t[:],
                                start=True,
                                stop=True,
                            )
                        u16 = u16p.tile([128, 512], f16)
                        copy_on(u_drain[n_ud % len(u_drain)], u16[:], u_ps[:])
                        n_ud += 1

                        z_ps = psz.tile([128, 512], f32)
                        for q in range(4):
                            nc.tensor.matmul(
                                z_ps[:, q * 128 : (q + 1) * 128],
                                u16[:, q * 128 : (q + 1) * 128],
                                dt1h_t[:],
                                start=True,
                                stop=True,
                            )
                        copy_on(
                            z_drain[n_zd % len(z_drain)],
                            zt[:, tjg * 512 : (tjg + 1) * 512],
                            z_ps[:],
                        )
                        n_zd += 1

                    e = out_engines[n_out % len(out_engines)]
                    n_out += 1
                    eng(e).dma_start(zraw[img, ti], zt[:])

    nc.compile()
    return nc


def _get_nc(n_img: int):
    if n_img not in _NC_CACHE:
        _NC_CACHE[n_img] = build_nc(n_img)
    return _NC_CACHE[n_img]


def make_inputs(x_core: np.ndarray, dt1h: np.ndarray) -> dict:
    return {"x": x_core, "dt1h": dt1h}


def _unpack(zraw: np.ndarray) -> np.ndarray:
    """zraw[img, ti, 16u+bi, tjg*512+q*128+16v+bj] -> out[img, 8u+v, h, w]."""
    n = zraw.shape[0]
    z = zraw.reshape(n, 8, 8, 16, 2, 4, 8, 16)  # img ti u bi tjg q v bj
    out = z.transpose(0, 2, 6, 1, 3, 4, 5, 7)  # img u v ti bi tjg q bj
    return out.astype(np.float32).reshape(n, 64, 128, 128)


def run_spmd(x: np.ndarray, dct: np.ndarray, trace: bool = False, nc=None):
    """Run the SPMD kernel on 8 cores. Returns (out, BassKernelResults)."""
    from concourse.bass_utils import run_bass_kernel_spmd

    x = np.asarray(x)
    if x.dtype != np.float16:
        x = x.astype(np.float16)
    x = np.ascontiguousarray(x)
    dct = np.asarray(dct, dtype=np.float32)
    b = x.shape[0]
    per = b // _N_CORES

    dt1h = _build_dt1(dct).astype(np.float16)

    if nc is None:
        nc = _get_nc(per)
    in_maps = [
        make_inputs(x[i * per : (i + 1) * per], dt1h) for i in range(_N_CORES)
    ]
    res = run_bass_kernel_spmd(
        nc, in_maps, core_ids=list(range(_N_CORES)), trace=trace
    )
    zraw = np.concatenate(
        [res.results[i]["zraw"] for i in range(_N_CORES)], axis=0
    )
    return _unpack(zraw), res


def kernel(x, dct=None):
    if dct is None:
        dct = _dct_mat_np()
    out, _ = run_spmd(x, dct, trace=False)
    return out


# revision 3
# speedup vs baseline: 2.2155x; 1.2194x over previous
"""8x8 block DCT (DCT-II) on [64,1,1024,1024] fp32 -> [64,64,128,128].

Data parallel over batch: 8 images per NeuronCore on 8 cores.

Accuracy budget is rel_err < 2e-2 (vs max|out|), so the pipeline runs in
fp16 (measured rel err ~4e-4). The host pre-flattens each 8x8 block into
a 64-vector (pure data marshalling: transpose + fp16 cast), which lets
the device compute the whole 2D DCT as a SINGLE fp16 matmul per
512-block chunk against the constant kron(M,M) matrix:

    zk[64a + 8u+v, n] = sum_k K128[k, 64a + 8u+v] * xk[k, n]
    K128 = blockdiag(K64, K64),  K64[8x+y, 8u+v] = M[u,x] M[v,y]

Two images ride the 128 partitions per matmul (a = image-in-pair), and
the result lands channel-major: partition p = 64a + ch, free n =
h*128 + w. PSUM is drained straight to fp16 SBUF (one drain per chunk,
alternating scalar/vector engines) and DMA'd out contiguously (2KB runs
per partition). The host upcasts + reshapes the output (no arithmetic).

DMA queues: input and output are spread across all three DGE paths
(gpsimd SW-DGE, sync HW-DGE, scalar HW-DGE) to run them in parallel.
"""

import numpy as np
from concurrent.futures import ThreadPoolExecutor

_N_CORES = 8
_H = 1024
_W = 1024

_NC_CACHE = {}

# tuning knobs
IN_ENGINES = "gcs"  # per 2MB input half-pair DMA
OUT_ENGINES = "sgc"  # per 512KB output DMA
DRAIN_ENGINES = "cv"  # per chunk PSUM->SBUF fp16 drain
XK_BUFS = 2
ZT_BUFS = 3
PS_BUFS = 4


def _dct_mat_np():
    n = 8
    u = np.arange(n)[:, None].astype(np.float64)
    x = np.arange(n)[None, :].astype(np.float64)
    m = np.cos((2 * x + 1) * u * np.pi / (2 * n))
    scale = np.where(u == 0, np.sqrt(1.0 / n), np.sqrt(2.0 / n))
    return (m * scale).astype(np.float32)


def _build_k128(dct: np.ndarray) -> np.ndarray:
    """K128[64a + 8x+y, 64a + 8u+v] = dct[u,x]*dct[v,y]."""
    k64 = np.einsum("ux,vy->xyuv", dct, dct).reshape(64, 64)
    k128 = np.zeros((128, 128), dtype=np.float32)
    k128[:64, :64] = k64
    k128[64:, 64:] = k64
    return k128


def build_nc(
    n_pair: int,
    in_engines=IN_ENGINES,
    out_engines=OUT_ENGINES,
    drain_engines=DRAIN_ENGINES,
    xk_bufs=XK_BUFS,
    zt_bufs=ZT_BUFS,
    ps_bufs=PS_BUFS,
):
    import concourse.bacc as bacc
    import concourse.mybir as mybir
    import concourse.tile as tile

    f32 = mybir.dt.float32
    f16 = mybir.dt.float16
    nc = bacc.Bacc("TRN2", target_bir_lowering=False, debug=False)

    xk = nc.dram_tensor("xk", [n_pair, 128, 16384], f16, kind="ExternalInput")
    k128 = nc.dram_tensor("k128", [128, 128], f16, kind="ExternalInput")
    zraw = nc.dram_tensor(
        "zraw", [n_pair, 8, 128, 2048], f16, kind="ExternalOutput"
    )

    def eng(ch):
        return {"s": nc.sync, "c": nc.scalar, "g": nc.gpsimd}[ch]

    def copy_on(ch, dst, src):
        if ch == "v":
            nc.vector.tensor_copy(dst, src)
        elif ch == "g":
            nc.gpsimd.tensor_copy(dst, src)
        else:
            eng(ch).copy(dst, src)

    n_in = 0
    n_out = 0
    n_dr = 0

    with tile.TileContext(nc) as tc:
        with (
            tc.tile_pool(name="const", bufs=1) as constp,
            tc.tile_pool(name="xk", bufs=xk_bufs) as xkp,
            tc.tile_pool(name="zt", bufs=zt_bufs) as ztp,
            tc.tile_pool(name="ps", bufs=ps_bufs, space="PSUM") as psp,
        ):
            k128_t = constp.tile([128, 128], f16)
            nc.sync.dma_start(k128_t[:], k128[:])

            for ip in range(n_pair):
                xk_t = xkp.tile([128, 16384], f16)
                for half in range(2):
                    e = in_engines[n_in % len(in_engines)]
                    n_in += 1
                    eng(e).dma_start(
                        xk_t[:, half * 8192 : (half + 1) * 8192],
                        xk[ip, :, half * 8192 : (half + 1) * 8192],
                    )

                for c4 in range(8):
                    zt = ztp.tile([128, 2048], f16)
                    for hh in range(4):
                        chunk = 4 * c4 + hh
                        ps = psp.tile([128, 512], f32)
                        nc.tensor.matmul(
                            ps[:],
                            k128_t[:],
                            xk_t[:, chunk * 512 : (chunk + 1) * 512],
                            start=True,
                            stop=True,
                        )
                        copy_on(
                            drain_engines[n_dr % len(drain_engines)],
                            zt[:, hh * 512 : (hh + 1) * 512],
                            ps[:],
                        )
                        n_dr += 1
                    e = out_engines[n_out % len(out_engines)]
                    n_out += 1
                    eng(e).dma_start(zraw[ip, c4], zt[:])

    nc.compile()
    return nc


def _get_nc(n_pair: int):
    if n_pair not in _NC_CACHE:
        _NC_CACHE[n_pair] = build_nc(n_pair)
    return _NC_CACHE[n_pair]


def _pmap(fn, n, workers=16):
    with ThreadPoolExecutor(workers) as ex:
        list(ex.map(fn, range(n)))


def _prep_x(x: np.ndarray) -> np.ndarray:
    """[B,1,1024,1024] f32 -> [B, 64, 16384] f16 block-flattened."""
    b = x.shape[0]
    src = x.reshape(b, 128, 8, 128, 8)
    out = np.empty((b, 8, 8, 128, 128), dtype=np.float16)

    def do(i):
        out[i] = src[i].transpose(1, 3, 0, 2)

    _pmap(do, b)
    return out.reshape(b, 64, 16384)


def _unpack(zraw: np.ndarray) -> np.ndarray:
    """zraw[ip, c4, 64a+ch, col4] -> out[img, ch, h, w] f32."""
    np_ = zraw.shape[0]  # total image pairs
    z = zraw.reshape(np_, 8, 2, 64, 2048)  # ip c4 a ch col4
    out = np.empty((np_, 2, 64, 8, 2048), dtype=np.float32)

    def do(i):
        out[i] = z[i].transpose(1, 2, 0, 3)

    _pmap(do, np_)
    return out.reshape(np_ * 2, 64, 128, 128)


def run_spmd(x: np.ndarray, dct: np.ndarray, trace: bool = False, nc=None):
    """Run the SPMD kernel on 8 cores. Returns (out, BassKernelResults)."""
    from concourse.bass_utils import run_bass_kernel_spmd

    x = np.asarray(x)
    dct = np.asarray(dct, dtype=np.float32)
    b = x.shape[0]
    per = b // _N_CORES  # images per core
    n_pair = per // 2

    xk_all = _prep_x(x)  # [B, 64, 16384] f16
    k128 = _build_k128(dct).astype(np.float16)

    if nc is None:
        nc = _get_nc(n_pair)
    in_maps = [
        {
            "xk": xk_all[i * per : (i + 1) * per].reshape(n_pair, 128, 16384),
            "k128": k128,
        }
        for i in range(_N_CORES)
    ]
    res = run_bass_kernel_spmd(
        nc, in_maps, core_ids=list(range(_N_CORES)), trace=trace
    )
    zraw = np.concatenate(
        [res.results[i]["zraw"] for i in range(_N_CORES)], axis=0
    )
    return _unpack(zraw), res


def kernel(x, dct=None):
    if dct is None:
        dct = _dct_mat_np()
    out, _ = run_spmd(x, dct, trace=False)
    return out


# revision 9
# speedup vs baseline: 2.3653x; 1.0676x over previous
"""8x8 block DCT (DCT-II) on [64,1,1024,1024] fp32 -> [64,64,128,128].

Data parallel over batch: 8 images per NeuronCore on 8 cores.

Accuracy budget is rel_err < 2e-2 (vs max|out|), so the pipeline runs in
fp16 (measured rel err ~4e-4). The host pre-flattens each 8x8 block into
a 64-vector (pure data marshalling: transpose + fp16 cast), which lets
the device compute the whole 2D DCT as a SINGLE fp16 matmul per
512-block chunk against the constant kron(M,M) matrix:

    zk[64a + 8u+v, n] = sum_k K128[k, 64a + 8u+v] * xk[k, n]
    K128 = blockdiag(K64, K64),  K64[8x+y, 8u+v] = M[u,x] M[v,y]

Two images ride the 128 partitions per matmul (a = image-in-pair), and
the result lands channel-major: partition p = 64a + ch, free n =
h*128 + w. PSUM is drained straight to fp16 SBUF (one drain per chunk,
alternating scalar/vector engines) and DMA'd out contiguously (2KB runs
per partition). The host upcasts + reshapes the output (no arithmetic).

DMA queues: input and output are spread across all three DGE paths
(gpsimd SW-DGE, sync HW-DGE, scalar HW-DGE) to run them in parallel.
"""

import numpy as np
from concurrent.futures import ThreadPoolExecutor

_N_CORES = 8
_H = 1024
_W = 1024

_NC_CACHE = {}

# tuning knobs
IN_ENGINES = "gcs"  # per input piece DMA
OUT_ENGINES = "sgc"  # per 512KB output DMA
DRAIN_ENGINES = "cv"  # per chunk PSUM->SBUF fp16 drain
IN_PIECES = 4  # input DMAs per image pair (finer => earlier first matmul)
ZT_BUFS = 3
PS_BUFS = 4


def _dct_mat_np():
    n = 8
    u = np.arange(n)[:, None].astype(np.float64)
    x = np.arange(n)[None, :].astype(np.float64)
    m = np.cos((2 * x + 1) * u * np.pi / (2 * n))
    scale = np.where(u == 0, np.sqrt(1.0 / n), np.sqrt(2.0 / n))
    return (m * scale).astype(np.float32)


def _build_k128(dct: np.ndarray) -> np.ndarray:
    """K128[64a + 8x+y, 64a + 8u+v] = dct[u,x]*dct[v,y]."""
    k64 = np.einsum("ux,vy->xyuv", dct, dct).reshape(64, 64)
    k128 = np.zeros((128, 128), dtype=np.float32)
    k128[:64, :64] = k64
    k128[64:, 64:] = k64
    return k128


def build_nc(
    n_pair: int,
    in_engines=IN_ENGINES,
    out_engines=OUT_ENGINES,
    drain_engines=DRAIN_ENGINES,
    in_pieces=IN_PIECES,
    zt_bufs=ZT_BUFS,
    ps_bufs=PS_BUFS,
):
    import concourse.bacc as bacc
    import concourse.mybir as mybir
    import concourse.tile as tile

    f32 = mybir.dt.float32
    f16 = mybir.dt.float16
    nc = bacc.Bacc("TRN2", target_bir_lowering=False, debug=False)

    xk = nc.dram_tensor("xk", [n_pair, 128, 16384], f16, kind="ExternalInput")
    k128 = nc.dram_tensor("k128", [128, 128], f16, kind="ExternalInput")
    zraw = nc.dram_tensor(
        "zraw", [n_pair, 8, 128, 2048], f16, kind="ExternalOutput"
    )

    def eng(ch):
        return {"s": nc.sync, "c": nc.scalar, "g": nc.gpsimd}[ch]

    def copy_on(ch, dst, src):
        if ch == "v":
            nc.vector.tensor_copy(dst, src)
        elif ch == "g":
            nc.gpsimd.tensor_copy(dst, src)
        else:
            eng(ch).copy(dst, src)

    n_in = 0
    n_out = 0
    n_dr = 0

    with tile.TileContext(nc) as tc:
        with (
            tc.tile_pool(name="const", bufs=1) as constp,
            tc.tile_pool(name="xk", bufs=1) as xkp,
            tc.tile_pool(name="zt", bufs=zt_bufs) as ztp,
            tc.tile_pool(name="ps", bufs=ps_bufs, space="PSUM") as psp,
        ):
            k128_t = constp.tile([128, 128], f16)
            nc.sync.dma_start(k128_t[:], k128[:])

            # all input DMAs issued upfront so the queues stream
            # continuously and no trigger queues behind drain copies
            xk_tiles = [
                xkp.tile([128, 16384], f16, name=f"xk{i}")
                for i in range(n_pair)
            ]
            pw = 16384 // in_pieces
            for ip in range(n_pair):
                for piece in range(in_pieces):
                    e = in_engines[n_in % len(in_engines)]
                    n_in += 1
                    eng(e).dma_start(
                        xk_tiles[ip][:, piece * pw : (piece + 1) * pw],
                        xk[ip, :, piece * pw : (piece + 1) * pw],
                    )

            for ip in range(n_pair):
                xk_t = xk_tiles[ip]
                for c4 in range(8):
                    zt = ztp.tile([128, 2048], f16)
                    for hh in range(4):
                        chunk = 4 * c4 + hh
                        ps = psp.tile([128, 512], f32)
                        nc.tensor.matmul(
                            ps[:],
                            k128_t[:],
                            xk_t[:, chunk * 512 : (chunk + 1) * 512],
                            start=True,
                            stop=True,
                        )
                        copy_on(
                            drain_engines[n_dr % len(drain_engines)],
                            zt[:, hh * 512 : (hh + 1) * 512],
                            ps[:],
                        )
                        n_dr += 1
                    e = out_engines[n_out % len(out_engines)]
                    n_out += 1
                    eng(e).dma_start(zraw[ip, c4], zt[:])

    nc.compile()
    return nc


def _get_nc(n_pair: int):
    if n_pair not in _NC_CACHE:
        _NC_CACHE[n_pair] = build_nc(n_pair)
    return _NC_CACHE[n_pair]


def _pmap(fn, n, workers=16):
    with ThreadPoolExecutor(workers) as ex:
        list(ex.map(fn, range(n)))


def _prep_x(x: np.ndarray) -> np.ndarray:
    """[B,1,1024,1024] f32 -> [B, 64, 16384] f16 block-flattened."""
    b = x.shape[0]
    src = x.reshape(b, 128, 8, 128, 8)
    out = np.empty((b, 8, 8, 128, 128), dtype=np.float16)

    def do(i):
        out[i] = src[i].transpose(1, 3, 0, 2)

    _pmap(do, b)
    return out.reshape(b, 64, 16384)


def _unpack(zraw: np.ndarray) -> np.ndarray:
    """zraw[ip, c4, 64a+ch, col4] -> out[img, ch, h, w] f32."""
    np_ = zraw.shape[0]  # total image pairs
    z = zraw.reshape(np_, 8, 2, 64, 2048)  # ip c4 a ch col4
    out = np.empty((np_, 2, 64, 8, 2048), dtype=np.float32)

    def do(i):
        out[i] = z[i].transpose(1, 2, 0, 3)

    _pmap(do, np_)
    return out.reshape(np_ * 2, 64, 128, 128)


def run_spmd(x: np.ndarray, dct: np.ndarray, trace: bool = False, nc=None):
    """Run the SPMD kernel on 8 cores. Returns (out, BassKernelResults)."""
    from concourse.bass_utils import run_bass_kernel_spmd

    x = np.asarray(x)
    dct = np.asarray(dct, dtype=np.float32)
    b = x.shape[0]
    per = b // _N_CORES  # images per core
    n_pair = per // 2

    xk_all = _prep_x(x)  # [B, 64, 16384] f16
    k128 = _build_k128(dct).astype(np.float16)

    if nc is None:
        nc = _get_nc(n_pair)
    in_maps = [
        {
            "xk": xk_all[i * per : (i + 1) * per].reshape(n_pair, 128, 16384),
            "k128": k128,
        }
        for i in range(_N_CORES)
    ]
    res = run_bass_kernel_spmd(
        nc, in_maps, core_ids=list(range(_N_CORES)), trace=trace
    )
    zraw = np.concatenate(
        [res.results[i]["zraw"] for i in range(_N_CORES)], axis=0
    )
    return _unpack(zraw), res


def kernel(x, dct=None):
    if dct is None:
        dct = _dct_mat_np()
    out, _ = run_spmd(x, dct, trace=False)
    return out
